# revision 10
# baseline (speedup 1.0000x reference)
"""Trainium2 Bass kernel for the e3nn-style 5x5x5 SAME conv (dense_cnn).

Strategy
--------
Data-parallel: 8 shards = 2 batches x 4 x-slabs of 12 output planes each.
Each core gets a zero/halo-padded, channel-first input slab and produces
[64, 12, 48, 48].

Sparsity-aware tap packing: the radial embedding has finite support, so
only taps with |offset| <= 2.24 (plus a tiny 2.45 ring) are nonzero --
57 of 125 taps. On-device the conv is a sum of 39 (not 75) PSUM-
accumulated matmuls per output tile:
  - SBUF plane tiles are "dup" tiles [128, 52*52]:
      partitions 0-63   = channel ch at voxel z
      partitions 64-127 = channel ch at voxel z+1
    with the z axis stored de-interleaved as (parity, half) so a matmul
    rhs can start at ANY z offset c with a contiguous column slice.
  - A matmul with K=128 applies weight blocks (s, q): out parity q at
    col n (z_out = 2n+q) accumulates k[tx, ty, c+s-q] -- covering taps
    {c-1, c, c+1} per matmul. Per (tx, ty) the set of needed z-taps is
    covered with 3 (center columns), 2 (r_xy in {1.41, 2}) or 1
    (r_xy = 2.24) matmuls; the 4 corner columns are entirely zero.
    39 matmuls/tile is provably minimal for this matmul family.
  - Dropping the half-covered slivers of the r=2.449 ring costs 4.1e-3
    relative error (gate is 2e-2).

The tiny 5x5x5x64x64 kernel build (radial basis x Clebsch-Gordan) is done
on the host in numpy and shipped as a packed [128, 39*128] weight input,
replicated to every core.
"""

import math

import numpy as np

import concourse.bass as bass
import concourse.mybir as mybir


def _np_mm_dtype():
    import ml_dtypes
    return {"float32r": np.float32, "float16": np.float16,
            "bfloat16": ml_dtypes.bfloat16}[MM_DTYPE]
from concourse import bacc, bass_utils
from concourse.tile import TileContext

MUL = 16
NB = 4
R = 2.5

N_CORES = 8
MM_DTYPE = "bfloat16"             # matmul operand dtype: float32r|float16|bfloat16
PX, PY, PZ = 16, 52, 52          # padded per-core input slab (x, y, z)
OX, OY, OZ = 12, 48, 48          # per-core output region
HPL = PZ // 2                    # 26 z-halves per parity block
PLANE2 = PY * PZ                 # 2704 cols per dup plane (y major, z=(par,h))
OPLANE = OY * OZ                 # 2304 outputs per x-plane
YB = 3                           # y-blocks of 16 rows -> N = 16*24 = 384
YBS = OY // YB


def _entries():
    """(tx, ty, c) matmul list; tap tz(s, q) = c + s - q."""
    out = []
    for tx in range(5):
        for ty in range(5):
            d2 = (tx - 2) ** 2 + (ty - 2) ** 2
            if d2 in (0, 1):
                cs = (0, 2, 4)
            elif d2 in (2, 4):
                cs = (1, 3)
            elif d2 == 5:
                cs = (2,)
            else:
                cs = ()
            for c in cs:
                out.append((tx, ty, c))
    return out

ENTRIES = _entries()
N_W = len(ENTRIES)               # 39
TX_COUNTS = [sum(1 for e in ENTRIES if e[0] == tx) for tx in range(5)]


def _build_k(w000, w011, w101, w110, sc0, sc1):
    """Numpy port of the reference kernel build. Returns [5,5,5,64,64]."""
    s = 2
    c = np.arange(-s, s + 1.0)
    lat = np.stack(np.meshgrid(c, c, c, indexing='ij'), axis=-1)
    norm = np.linalg.norm(lat, axis=-1)
    safe = np.where(norm == 0.0, 1.0, norm)
    nvec = np.where(norm[..., None] > 0.0, lat / safe[..., None], 0.0)
    sh1 = np.sqrt(3.0) * nvec
    values = np.linspace(0.0, R, NB + 2)[1:-1]
    step = R / (NB + 1)
    d = (norm[..., None] - values) / step
    dd = np.clip(d, -1.0 + 1e-9, 1.0 - 1e-9)
    emb = np.where(np.abs(d) < 1.0,
                   1.14136 * np.e ** 2 * np.exp(-1.0 / (1.0 - dd ** 2)), 0.0)
    nlat = 125.0

    r000 = np.einsum('xyzb,buw->xyzuw', emb, w000) / nlat
    r011 = np.einsum('xyzb,buw->xyzuw', emb, w011) / nlat
    r101 = np.einsum('xyzb,buw->xyzuw', emb, w101) / nlat
    r110 = np.einsum('xyzb,buw->xyzuw', emb, w110) / nlat
    eye3 = np.eye(3)
    k00 = r000
    k01 = np.einsum('xyzuw,xyzk->xyzuwk', r011, sh1).reshape(5, 5, 5, MUL, 3 * MUL)
    k11 = np.einsum('xyzuw,ik->xyzuiwk', r101, eye3).reshape(5, 5, 5, 3 * MUL, 3 * MUL)
    k10 = np.einsum('xyzuw,xyzi->xyzuiw', r110, sh1).reshape(5, 5, 5, 3 * MUL, MUL) / np.sqrt(3.0)
    top = np.concatenate([k00, k01], axis=-1)
    bot = np.concatenate([k10, k11], axis=-1)
    k = np.concatenate([top, bot], axis=-2)

    lin00 = sc0 / np.sqrt(float(MUL))
    lin11 = np.einsum('uw,ik->uiwk', sc1 / np.sqrt(float(MUL)), eye3).reshape(3 * MUL, 3 * MUL)
    z16 = np.zeros((MUL, 3 * MUL))
    lin = np.concatenate([
        np.concatenate([lin00, z16], axis=1),
        np.concatenate([z16.T, lin11], axis=1)], axis=0)
    k[2, 2, 2] = lin
    return k


def _pack_weights(k):
    """[128, 39*128] with W[s*64+ci, i*128 + q*64+co] = k[tx,ty,c+s-q]."""
    Ws = np.zeros((N_W, 128, 128))
    for i, (tx, ty, c) in enumerate(ENTRIES):
        for s in range(2):
            for q in range(2):
                tz = c + s - q
                if 0 <= tz <= 4:
                    Ws[i, s * 64:(s + 1) * 64, q * 64:(q + 1) * 64] = k[tx, ty, tz]
    return np.ascontiguousarray(
        Ws.transpose(1, 0, 2).reshape(128, N_W * 128)).astype(_np_mm_dtype())


_NC = None


def _get_nc():
    global _NC
    if _NC is None:
        _NC = _build_nc()
    return _NC


def _build_nc():
    nc = bacc.Bacc("TRN2", target_bir_lowering=False)
    f32 = mybir.dt.float32
    fmm = getattr(mybir.dt, MM_DTYPE)

    xin = nc.dram_tensor("xin", [128, PX * PLANE2], fmm, kind="ExternalInput")
    wts = nc.dram_tensor("wts", [128, N_W * 128], fmm, kind="ExternalInput")
    yout = nc.dram_tensor("yout", [64, OX * OPLANE], f32, kind="ExternalOutput")

    with TileContext(nc) as tc:
        with tc.tile_pool(name="wpool", bufs=1) as wpool, \
             tc.tile_pool(name="xpool", bufs=8) as xpool, \
             tc.tile_pool(name="opool", bufs=2) as opool, \
             tc.tile_pool(name="ppool", bufs=6, space="PSUM") as ppool:

            planes = {}

            def get_plane(px):
                # dup plane tile: col (y, par, h); partitions 0-63 hold
                # ch at z=2h+par, 64-127 hold z+1.  The first planes are
                # loaded in two halves so the first y-block's matmuls can
                # start as soon as rows 0-27 land.
                if px not in planes:
                    pt = xpool.tile([128, PLANE2], fmm, tag="plane", name="plane")
                    base = px * PLANE2
                    if px < 3:
                        half = 28 * PZ
                        nc.sync.dma_start(out=pt[:, :half],
                                          in_=xin[:, base:base + half])
                        nc.sync.dma_start(out=pt[:, half:],
                                          in_=xin[:, base + half:base + PLANE2])
                    else:
                        nc.sync.dma_start(out=pt[:, :],
                                          in_=xin[:, base:base + PLANE2])
                    planes[px] = pt
                return planes[px]

            # interleave weight-chunk and plane DMA issue so the first
            # matmuls' dependencies (chunk 0, plane 0, 1, ...) land first
            wt_chunks = [None] * 5
            wt_offs = [sum(TX_COUNTS[:t]) for t in range(5)]

            def load_chunk(txc):
                n = TX_COUNTS[txc]
                wtc = wpool.tile([128, n * 128], fmm, tag="wt", bufs=5,
                                 name="wt")
                off = wt_offs[txc]
                nc.sync.dma_start(out=wtc[:, :],
                                  in_=wts[:, off * 128:(off + n) * 128])
                wt_chunks[txc] = wtc

            # plane 0's first half (rows 0-27, all the first y-block
            # needs) and weight chunk 0 land before anything else, so the
            # first matmuls' dependencies clear as early as possible
            pt0 = xpool.tile([128, PLANE2], fmm, tag="plane", name="plane")
            half0 = 28 * PZ
            nc.sync.dma_start(out=pt0[:, :half0], in_=xin[:, :half0])
            planes[0] = pt0
            load_chunk(0)
            nc.sync.dma_start(out=pt0[:, half0:], in_=xin[:, half0:PLANE2])
            get_plane(1)
            load_chunk(1)
            get_plane(2)
            load_chunk(2)
            load_chunk(3)
            load_chunk(4)
            get_plane(3)
            get_plane(4)
            get_plane(5)

            # warm-up: ~16 dummy matmuls on a zeroed scratch tile ramp the
            # PE clock (0.65->2.4 GHz after ~3us busy) while the first
            # input DMAs are still in flight
            warm = wpool.tile([128, 128], fmm, tag="warm", bufs=1,
                              name="warm")
            nc.any.memset(warm[:, :], 0)
            wps = ppool.tile([128, 512], f32, tag="wps", bufs=1, name="wps")
            for _ in range(54):
                nc.tensor.matmul(wps[:, :128], warm[:, :], warm[:, :],
                                 start=True, stop=True)

            # per-tx entry sublists with chunk-local weight index
            tx_entries = []
            base = 0
            for txc in range(5):
                sub = [(j, e[1], e[2]) for j, e in
                       enumerate(ENTRIES[base:base + TX_COUNTS[txc]])]
                tx_entries.append(sub)
                base += TX_COUNTS[txc]

            def copy_out(ostv, psv, y0, q):
                # alternate DVE / ACT so the two per-bank copies drain in
                # parallel instead of serializing on Vector
                dst = ostv[:, y0:y0 + YBS, q:OZ:2]
                src = psv[q * 64:(q + 1) * 64, :, :]
                if q == 0:
                    nc.vector.tensor_copy(dst, src)
                else:
                    nc.scalar.activation(
                        dst, src, mybir.ActivationFunctionType.Copy)

            for xo in range(OX):
                ostage = opool.tile([64, OPLANE], f32, name="ostage")
                ostv = ostage.rearrange("c (y z) -> c y z", z=OZ)
                last = xo == OX - 1
                if not last:
                    # full-bank PSUM tiles (512 f32): partial-bank tiles
                    # measured +25ns/matmul on the PE stream pace.  All 3
                    # y-blocks accumulate concurrently so 3 consecutive
                    # matmuls share one stationary weight load.
                    pss = [ppool.tile([128, 512], f32, name="ps")
                           for _ in range(YB)]
                    i = 0
                    for txc in range(5):
                        pt = get_plane(xo + txc)
                        ptv = pt.rearrange("c (y z) -> c y z", z=PZ)
                        for (j, ty, c) in tx_entries[txc]:
                            zoff = (c & 1) * HPL + (c >> 1)
                            lhsT = wt_chunks[txc][:, j * 128:(j + 1) * 128]
                            for yb in range(YB):
                                y0 = yb * YBS
                                rhs = ptv[:, y0 + ty:y0 + ty + YBS,
                                          zoff:zoff + OZ // 2]
                                ps = pss[yb][:, :YBS * (OZ // 2)]
                                nc.tensor.matmul(ps[:, :], lhsT, rhs,
                                                 start=(i == 0),
                                                 stop=(i == N_W - 1))
                            i += 1
                    for yb in range(YB):
                        y0 = yb * YBS
                        ps = pss[yb][:, :YBS * (OZ // 2)]
                        psv = ps.rearrange("c (y z) -> c y z", z=OZ // 2)
                        for q in range(2):
                            copy_out(ostv, psv, y0, q)
                    nc.sync.dma_start(
                        out=yout[:, xo * OPLANE:(xo + 1) * OPLANE],
                        in_=ostage[:, :])
                else:
                    # last plane: sequential per-yb groups so yb0/yb1
                    # copies and output DMAs overlap yb1/yb2 matmuls,
                    # shrinking the post-stream tail
                    for yb in range(YB):
                        y0 = yb * YBS
                        ps_full = ppool.tile([128, 512], f32, name="ps")
                        ps = ps_full[:, :YBS * (OZ // 2)]
                        i = 0
                        for txc in range(5):
                            pt = get_plane(xo + txc)
                            ptv = pt.rearrange("c (y z) -> c y z", z=PZ)
                            for (j, ty, c) in tx_entries[txc]:
                                zoff = (c & 1) * HPL + (c >> 1)
                                lhsT = wt_chunks[txc][:, j * 128:(j + 1) * 128]
                                rhs = ptv[:, y0 + ty:y0 + ty + YBS,
                                          zoff:zoff + OZ // 2]
                                nc.tensor.matmul(ps[:, :], lhsT, rhs,
                                                 start=(i == 0),
                                                 stop=(i == N_W - 1))
                                i += 1
                        psv = ps.rearrange("c (y z) -> c y z", z=OZ // 2)
                        for q in range(2):
                            copy_out(ostv, psv, y0, q)
                        nc.sync.dma_start(
                            out=yout[:, xo * OPLANE + y0 * OZ:
                                     xo * OPLANE + (y0 + YBS) * OZ],
                            in_=ostage[:, y0 * OZ:(y0 + YBS) * OZ])
    nc.finalize()
    return nc


def _prep_inputs(x, wts_arr):
    """Returns per-core in_maps. x: [2,48,48,48,64] float32."""
    in_maps = []
    for core in range(N_CORES):
        n, xs = core // 4, (core % 4) * OX
        xpadn = np.pad(x[n], ((2, 2), (2, 2), (2, 2), (0, 0)))
        slab = xpadn[xs:xs + PX]                               # [16,52,52,64]
        xc = slab.transpose(3, 0, 1, 2).astype(_np_mm_dtype())  # [64,16,52,52]
        xsh = np.zeros_like(xc)
        xsh[..., :-1] = xc[..., 1:]                            # z+1 shift
        dup = np.concatenate([xc, xsh], axis=0)                # [128,16,52,52]
        # z -> (parity, half) de-interleave for contiguous rhs slices
        dup = np.stack([dup[..., 0::2], dup[..., 1::2]], axis=3)
        in_maps.append({
            "xin": np.ascontiguousarray(dup).reshape(128, PX * PLANE2),
            "wts": wts_arr,
        })
    return in_maps


def _run(inputs, trace=False):
    x = np.asarray(inputs["x"], np.float32)
    k = _build_k(np.asarray(inputs["w000"], np.float64),
                 np.asarray(inputs["w011"], np.float64),
                 np.asarray(inputs["w101"], np.float64),
                 np.asarray(inputs["w110"], np.float64),
                 np.asarray(inputs["sc0"], np.float64),
                 np.asarray(inputs["sc1"], np.float64))
    wts_arr = _pack_weights(k)
    in_maps = _prep_inputs(x, wts_arr)

    nc = _get_nc()
    res = bass_utils.run_bass_kernel_spmd(
        nc, in_maps, core_ids=list(range(N_CORES)), trace=trace)

    out = np.empty((2, 48, 48, 48, 64), np.float32)
    for core in range(N_CORES):
        n, xs = core // 4, (core % 4) * OX
        oc = res.results[core]["yout"].reshape(64, OX, OY, OZ)
        out[n, xs:xs + OX] = oc.transpose(1, 2, 3, 0)
    return out, res


def kernel(**inputs):
    out, _ = _run(inputs, trace=False)
    return out


# revision 11
# speedup vs baseline: 1.0024x; 1.0024x over previous
"""Trainium2 Bass kernel for the e3nn-style 5x5x5 SAME conv (dense_cnn).

Strategy
--------
Data-parallel: 8 shards = 2 batches x 4 x-slabs of 12 output planes each.
Each core gets a zero/halo-padded, channel-first input slab and produces
[64, 12, 48, 48].

Sparsity-aware tap packing: the radial embedding has finite support, so
only taps with |offset| <= 2.24 (plus a tiny 2.45 ring) are nonzero --
57 of 125 taps. On-device the conv is a sum of 39 (not 75) PSUM-
accumulated matmuls per output tile:
  - SBUF plane tiles are "dup" tiles [128, 52*52]:
      partitions 0-63   = channel ch at voxel z
      partitions 64-127 = channel ch at voxel z+1
    with the z axis stored de-interleaved as (parity, half) so a matmul
    rhs can start at ANY z offset c with a contiguous column slice.
  - A matmul with K=128 applies weight blocks (s, q): out parity q at
    col n (z_out = 2n+q) accumulates k[tx, ty, c+s-q] -- covering taps
    {c-1, c, c+1} per matmul. Per (tx, ty) the set of needed z-taps is
    covered with 3 (center columns), 2 (r_xy in {1.41, 2}) or 1
    (r_xy = 2.24) matmuls; the 4 corner columns are entirely zero.
    39 matmuls/tile is provably minimal for this matmul family.
  - Dropping the half-covered slivers of the r=2.449 ring costs 4.1e-3
    relative error (gate is 2e-2).

The tiny 5x5x5x64x64 kernel build (radial basis x Clebsch-Gordan) is done
on the host in numpy and shipped as a packed [128, 39*128] weight input,
replicated to every core.
"""

import math

import numpy as np

import concourse.bass as bass
import concourse.mybir as mybir


def _np_mm_dtype():
    import ml_dtypes
    return {"float32r": np.float32, "float16": np.float16,
            "bfloat16": ml_dtypes.bfloat16}[MM_DTYPE]
from concourse import bacc, bass_utils
from concourse.tile import TileContext

MUL = 16
NB = 4
R = 2.5

N_CORES = 8
MM_DTYPE = "bfloat16"             # matmul operand dtype: float32r|float16|bfloat16
PX, PY, PZ = 16, 52, 52          # padded per-core input slab (x, y, z)
OX, OY, OZ = 12, 48, 48          # per-core output region
HPL = PZ // 2                    # 26 z-halves per parity block
PLANE2 = PY * PZ                 # 2704 cols per dup plane (y major, z=(par,h))
OPLANE = OY * OZ                 # 2304 outputs per x-plane
YB = 3                           # y-blocks of 16 rows -> N = 16*24 = 384
YBS = OY // YB


def _entries():
    """(tx, ty, c) matmul list; tap tz(s, q) = c + s - q."""
    out = []
    for tx in range(5):
        for ty in range(5):
            d2 = (tx - 2) ** 2 + (ty - 2) ** 2
            if d2 in (0, 1):
                cs = (0, 2, 4)
            elif d2 in (2, 4):
                cs = (1, 3)
            elif d2 == 5:
                cs = (2,)
            else:
                cs = ()
            for c in cs:
                out.append((tx, ty, c))
    return out

ENTRIES = _entries()
N_W = len(ENTRIES)               # 39
TX_COUNTS = [sum(1 for e in ENTRIES if e[0] == tx) for tx in range(5)]


def _build_k(w000, w011, w101, w110, sc0, sc1):
    """Numpy port of the reference kernel build. Returns [5,5,5,64,64]."""
    s = 2
    c = np.arange(-s, s + 1.0)
    lat = np.stack(np.meshgrid(c, c, c, indexing='ij'), axis=-1)
    norm = np.linalg.norm(lat, axis=-1)
    safe = np.where(norm == 0.0, 1.0, norm)
    nvec = np.where(norm[..., None] > 0.0, lat / safe[..., None], 0.0)
    sh1 = np.sqrt(3.0) * nvec
    values = np.linspace(0.0, R, NB + 2)[1:-1]
    step = R / (NB + 1)
    d = (norm[..., None] - values) / step
    dd = np.clip(d, -1.0 + 1e-9, 1.0 - 1e-9)
    emb = np.where(np.abs(d) < 1.0,
                   1.14136 * np.e ** 2 * np.exp(-1.0 / (1.0 - dd ** 2)), 0.0)
    nlat = 125.0

    r000 = np.einsum('xyzb,buw->xyzuw', emb, w000) / nlat
    r011 = np.einsum('xyzb,buw->xyzuw', emb, w011) / nlat
    r101 = np.einsum('xyzb,buw->xyzuw', emb, w101) / nlat
    r110 = np.einsum('xyzb,buw->xyzuw', emb, w110) / nlat
    eye3 = np.eye(3)
    k00 = r000
    k01 = np.einsum('xyzuw,xyzk->xyzuwk', r011, sh1).reshape(5, 5, 5, MUL, 3 * MUL)
    k11 = np.einsum('xyzuw,ik->xyzuiwk', r101, eye3).reshape(5, 5, 5, 3 * MUL, 3 * MUL)
    k10 = np.einsum('xyzuw,xyzi->xyzuiw', r110, sh1).reshape(5, 5, 5, 3 * MUL, MUL) / np.sqrt(3.0)
    top = np.concatenate([k00, k01], axis=-1)
    bot = np.concatenate([k10, k11], axis=-1)
    k = np.concatenate([top, bot], axis=-2)

    lin00 = sc0 / np.sqrt(float(MUL))
    lin11 = np.einsum('uw,ik->uiwk', sc1 / np.sqrt(float(MUL)), eye3).reshape(3 * MUL, 3 * MUL)
    z16 = np.zeros((MUL, 3 * MUL))
    lin = np.concatenate([
        np.concatenate([lin00, z16], axis=1),
        np.concatenate([z16.T, lin11], axis=1)], axis=0)
    k[2, 2, 2] = lin
    return k


def _pack_weights(k):
    """[128, 39*128] with W[s*64+ci, i*128 + q*64+co] = k[tx,ty,c+s-q]."""
    Ws = np.zeros((N_W, 128, 128))
    for i, (tx, ty, c) in enumerate(ENTRIES):
        for s in range(2):
            for q in range(2):
                tz = c + s - q
                if 0 <= tz <= 4:
                    Ws[i, s * 64:(s + 1) * 64, q * 64:(q + 1) * 64] = k[tx, ty, tz]
    return np.ascontiguousarray(
        Ws.transpose(1, 0, 2).reshape(128, N_W * 128)).astype(_np_mm_dtype())


_NC = None


def _get_nc():
    global _NC
    if _NC is None:
        _NC = _build_nc()
    return _NC


def _build_nc():
    nc = bacc.Bacc("TRN2", target_bir_lowering=False)
    f32 = mybir.dt.float32
    fmm = getattr(mybir.dt, MM_DTYPE)

    xin = nc.dram_tensor("xin", [128, PX * PLANE2], fmm, kind="ExternalInput")
    wts = nc.dram_tensor("wts", [128, N_W * 128], fmm, kind="ExternalInput")
    yout = nc.dram_tensor("yout", [64, OX * OPLANE], f32, kind="ExternalOutput")

    with TileContext(nc) as tc:
        with tc.tile_pool(name="wpool", bufs=1) as wpool, \
             tc.tile_pool(name="xpool", bufs=8) as xpool, \
             tc.tile_pool(name="opool", bufs=2) as opool, \
             tc.tile_pool(name="ppool", bufs=6, space="PSUM") as ppool:

            planes = {}

            def get_plane(px):
                # dup plane tile: col (y, par, h); partitions 0-63 hold
                # ch at z=2h+par, 64-127 hold z+1.  The first planes are
                # loaded in two halves so the first y-block's matmuls can
                # start as soon as rows 0-27 land.
                if px not in planes:
                    pt = xpool.tile([128, PLANE2], fmm, tag="plane", name="plane")
                    base = px * PLANE2
                    if px < 3:
                        half = 28 * PZ
                        nc.sync.dma_start(out=pt[:, :half],
                                          in_=xin[:, base:base + half])
                        nc.sync.dma_start(out=pt[:, half:],
                                          in_=xin[:, base + half:base + PLANE2])
                    else:
                        nc.sync.dma_start(out=pt[:, :],
                                          in_=xin[:, base:base + PLANE2])
                    planes[px] = pt
                return planes[px]

            # interleave weight-chunk and plane DMA issue so the first
            # matmuls' dependencies (chunk 0, plane 0, 1, ...) land first
            wt_chunks = [None] * 5
            wt_offs = [sum(TX_COUNTS[:t]) for t in range(5)]

            def load_chunk(txc):
                n = TX_COUNTS[txc]
                wtc = wpool.tile([128, n * 128], fmm, tag="wt", bufs=5,
                                 name="wt")
                off = wt_offs[txc]
                nc.sync.dma_start(out=wtc[:, :],
                                  in_=wts[:, off * 128:(off + n) * 128])
                wt_chunks[txc] = wtc

            get_plane(0)
            load_chunk(0)
            get_plane(1)
            load_chunk(1)
            get_plane(2)
            load_chunk(2)
            load_chunk(3)
            load_chunk(4)
            get_plane(3)
            get_plane(4)
            get_plane(5)

            # warm-up: ~16 dummy matmuls on a zeroed scratch tile ramp the
            # PE clock (0.65->2.4 GHz after ~3us busy) while the first
            # input DMAs are still in flight
            warm = wpool.tile([128, 128], fmm, tag="warm", bufs=1,
                              name="warm")
            nc.any.memset(warm[:, :], 0)
            wps = ppool.tile([128, 512], f32, tag="wps", bufs=1, name="wps")
            for _ in range(40):
                nc.tensor.matmul(wps[:, :128], warm[:, :], warm[:, :],
                                 start=True, stop=True)

            # per-tx entry sublists with chunk-local weight index
            tx_entries = []
            base = 0
            for txc in range(5):
                sub = [(j, e[1], e[2]) for j, e in
                       enumerate(ENTRIES[base:base + TX_COUNTS[txc]])]
                tx_entries.append(sub)
                base += TX_COUNTS[txc]

            def copy_out(ostv, psv, y0, q):
                # alternate DVE / ACT so the two per-bank copies drain in
                # parallel instead of serializing on Vector
                dst = ostv[:, y0:y0 + YBS, q:OZ:2]
                src = psv[q * 64:(q + 1) * 64, :, :]
                if q == 0:
                    nc.vector.tensor_copy(dst, src)
                else:
                    nc.scalar.activation(
                        dst, src, mybir.ActivationFunctionType.Copy)

            for xo in range(OX):
                ostage = opool.tile([64, OPLANE], f32, name="ostage")
                ostv = ostage.rearrange("c (y z) -> c y z", z=OZ)
                last = xo == OX - 1
                if not last:
                    # full-bank PSUM tiles (512 f32): partial-bank tiles
                    # measured +25ns/matmul on the PE stream pace.  All 3
                    # y-blocks accumulate concurrently so 3 consecutive
                    # matmuls share one stationary weight load.
                    pss = [ppool.tile([128, 512], f32, name="ps")
                           for _ in range(YB)]
                    i = 0
                    for txc in range(5):
                        pt = get_plane(xo + txc)
                        ptv = pt.rearrange("c (y z) -> c y z", z=PZ)
                        for (j, ty, c) in tx_entries[txc]:
                            zoff = (c & 1) * HPL + (c >> 1)
                            lhsT = wt_chunks[txc][:, j * 128:(j + 1) * 128]
                            for yb in range(YB):
                                y0 = yb * YBS
                                rhs = ptv[:, y0 + ty:y0 + ty + YBS,
                                          zoff:zoff + OZ // 2]
                                ps = pss[yb][:, :YBS * (OZ // 2)]
                                nc.tensor.matmul(ps[:, :], lhsT, rhs,
                                                 start=(i == 0),
                                                 stop=(i == N_W - 1))
                            i += 1
                    for yb in range(YB):
                        y0 = yb * YBS
                        ps = pss[yb][:, :YBS * (OZ // 2)]
                        psv = ps.rearrange("c (y z) -> c y z", z=OZ // 2)
                        for q in range(2):
                            copy_out(ostv, psv, y0, q)
                    nc.sync.dma_start(
                        out=yout[:, xo * OPLANE:(xo + 1) * OPLANE],
                        in_=ostage[:, :])
                else:
                    # last plane: sequential per-yb groups so yb0/yb1
                    # copies and output DMAs overlap yb1/yb2 matmuls,
                    # shrinking the post-stream tail
                    for yb in range(YB):
                        y0 = yb * YBS
                        ps_full = ppool.tile([128, 512], f32, name="ps")
                        ps = ps_full[:, :YBS * (OZ // 2)]
                        i = 0
                        for txc in range(5):
                            pt = get_plane(xo + txc)
                            ptv = pt.rearrange("c (y z) -> c y z", z=PZ)
                            for (j, ty, c) in tx_entries[txc]:
                                zoff = (c & 1) * HPL + (c >> 1)
                                lhsT = wt_chunks[txc][:, j * 128:(j + 1) * 128]
                                rhs = ptv[:, y0 + ty:y0 + ty + YBS,
                                          zoff:zoff + OZ // 2]
                                nc.tensor.matmul(ps[:, :], lhsT, rhs,
                                                 start=(i == 0),
                                                 stop=(i == N_W - 1))
                                i += 1
                        psv = ps.rearrange("c (y z) -> c y z", z=OZ // 2)
                        for q in range(2):
                            copy_out(ostv, psv, y0, q)
                        nc.sync.dma_start(
                            out=yout[:, xo * OPLANE + y0 * OZ:
                                     xo * OPLANE + (y0 + YBS) * OZ],
                            in_=ostage[:, y0 * OZ:(y0 + YBS) * OZ])
    nc.finalize()
    return nc


def _prep_inputs(x, wts_arr):
    """Returns per-core in_maps. x: [2,48,48,48,64] float32."""
    in_maps = []
    for core in range(N_CORES):
        n, xs = core // 4, (core % 4) * OX
        xpadn = np.pad(x[n], ((2, 2), (2, 2), (2, 2), (0, 0)))
        slab = xpadn[xs:xs + PX]                               # [16,52,52,64]
        xc = slab.transpose(3, 0, 1, 2).astype(_np_mm_dtype())  # [64,16,52,52]
        xsh = np.zeros_like(xc)
        xsh[..., :-1] = xc[..., 1:]                            # z+1 shift
        dup = np.concatenate([xc, xsh], axis=0)                # [128,16,52,52]
        # z -> (parity, half) de-interleave for contiguous rhs slices
        dup = np.stack([dup[..., 0::2], dup[..., 1::2]], axis=3)
        in_maps.append({
            "xin": np.ascontiguousarray(dup).reshape(128, PX * PLANE2),
            "wts": wts_arr,
        })
    return in_maps


def _run(inputs, trace=False):
    x = np.asarray(inputs["x"], np.float32)
    k = _build_k(np.asarray(inputs["w000"], np.float64),
                 np.asarray(inputs["w011"], np.float64),
                 np.asarray(inputs["w101"], np.float64),
                 np.asarray(inputs["w110"], np.float64),
                 np.asarray(inputs["sc0"], np.float64),
                 np.asarray(inputs["sc1"], np.float64))
    wts_arr = _pack_weights(k)
    in_maps = _prep_inputs(x, wts_arr)

    nc = _get_nc()
    res = bass_utils.run_bass_kernel_spmd(
        nc, in_maps, core_ids=list(range(N_CORES)), trace=trace)

    out = np.empty((2, 48, 48, 48, 64), np.float32)
    for core in range(N_CORES):
        n, xs = core // 4, (core % 4) * OX
        oc = res.results[core]["yout"].reshape(64, OX, OY, OZ)
        out[n, xs:xs + OX] = oc.transpose(1, 2, 3, 0)
    return out, res


def kernel(**inputs):
    out, _ = _run(inputs, trace=False)
    return out
